# revision 36
# baseline (speedup 1.0000x reference)
"""Trainium2 Bass kernel for dual-attention (DisKT-style) nn module.

Math per (batch, head) with S=1024, dk=64, all on-chip in [k, q] layout:
    sT       = (k_h @ q_h^T)            (+ -1e30 on causal-dead diag block)
    E1T      = exp(sT / 8)              (causally-dead region never computed)
    r1[q]    = sum_k E1T[k, q]          (ones^T @ E1T, PSUM broadcast rows)
    p1       = E1T * rec1[q]
    ET       = exp(p1)                  (bf16; exactly 1.0 at dead positions)
    outT     = (cm*v/S)^T @ ET + vcorr[chunk(q)]
    out[:, q=0] = 0

Key algebraic simplifications vs a direct translation:
  * The second softmax denominator r2 = S + sum_k cml*(exp(p1)-1) lies in
    [S, S+e-1] (since sum_k p1 <= 1), i.e. within 0.17% of S.  We use
    rec2 = 1/S exactly, folded into v host-side; error ~1e-3 << 2e-2 tol.
  * Weights are exp(p1) directly (no "-1" pass).  The exp(0)=1 dummy
    contribution of masked keys in chunks <= chunk(q) and of all keys in
    chunks > chunk(q) is a PER-CHUNK constant vector
        vcorr[c] = (sum_{k masked, chunk<=c} v_k + sum_{chunk>c} v_k)/S
    precomputed host-side and added per 128-wide q-chunk at the end.
    Dead in-diag positions have E=exp(0)=1 exactly (bf16), matching the
    masked-v PV term they need.

Live (causal) regions are stored PACKED so exp runs as few big
instructions; the counter-mask is folded into the PV weights host-side.
Emission is software-pipelined with a 2-block skew (A(n) | C(n-1) | D(n-2))
so the in-order PE stream never stalls on the exp chain.

Sharding: data-parallel over batch, B=16 -> 2 per core on 8 cores.
"""

import numpy as np
import ml_dtypes

import concourse.bass as bass
import concourse.mybir as mybir
import concourse.tile as tile
from concourse import bacc
from concourse.bass_utils import run_bass_kernel_spmd

B, S, D, H = 16, 1024, 512, 8
DK = D // H           # 64
NCORES = 8
BLOC = B // NCORES    # 2 batches per core
NCH = S // 128        # 8 k-chunks of 128
F32 = mybir.dt.float32
BF16 = mybir.dt.bfloat16
NPBF16 = ml_dtypes.bfloat16

LIVE = [S - 128 * c for c in range(NCH)]          # live width per chunk
OFF = [sum(LIVE[:c]) for c in range(NCH)]         # packed offset per chunk
PACK = OFF[-1] + LIVE[-1]                         # 4608
# chunk groups sharing one scores-psum tile + one exp1 instruction
GROUPS = [[0], [1], [2], [3], [4, 5], [6, 7]]
# which chunks' p1-multiply runs on DVE (rest on GpSimd)
MUL_ON_DVE = {0, 1}
# split the packed exp2 for latency (chunks 0-1 | 2-7)
CSPLIT = OFF[2]

# knobs that test.py can flip
TRACE = False
LAST_RESULTS = None


def build_nc(debug=False):
    nc = bacc.Bacc("TRN2", target_bir_lowering=False, debug=debug)
    AF = mybir.ActivationFunctionType

    qt_d = nc.dram_tensor("qt", [BLOC, H, DK, S], BF16, kind="ExternalInput")
    kt_d = nc.dram_tensor("kt", [BLOC, H, DK, S], BF16, kind="ExternalInput")
    # (1-cm)*[v1|v2]/S per (b, h), pre-transposed to [key-in-chunk, c*128+j]
    # so the DMA is contiguous 2KB per partition line
    vcat_d = nc.dram_tensor(
        "vcat", [BLOC, H, 128, NCH * 128], BF16, kind="ExternalInput"
    )
    # per-chunk constant correction (masked-prefix + future-chunk sums)/S,
    # pre-transposed host-side to [d, (b h c)]
    vcorr_d = nc.dram_tensor("vcorr", [128, BLOC * H * NCH], F32, kind="ExternalInput")
    dmask_d = nc.dram_tensor("dmask", [128, 128], BF16, kind="ExternalInput")
    ident_d = nc.dram_tensor("ident", [128, 128], BF16, kind="ExternalInput")
    ones_d = nc.dram_tensor("onesd", [128, 128], BF16, kind="ExternalInput")
    out1_d = nc.dram_tensor("out1t", [BLOC, D, S], F32, kind="ExternalOutput")
    out2_d = nc.dram_tensor("out2t", [BLOC, D, S], F32, kind="ExternalOutput")

    def bank_pieces(p0, p1):
        """split [p0, p1) at 512-aligned psum bank boundaries"""
        out = []
        p = p0
        while p < p1:
            end = min(p1, (p // 512 + 1) * 512)
            out.append((p, end))
            p = end
        return out

    with tile.TileContext(nc) as tc:
        with (
            tc.tile_pool(name="consts", bufs=1) as consts,
            tc.tile_pool(name="qk", bufs=3) as qkp,
            tc.tile_pool(name="vc", bufs=5) as vcp,
            tc.tile_pool(name="e1", bufs=3) as e1p,
            tc.tile_pool(name="e2", bufs=4) as e2p,
            tc.tile_pool(name="tmp", bufs=2) as tmpp,
            tc.tile_pool(name="rc", bufs=3) as rcp,
            tc.tile_pool(name="outs", bufs=2) as outp,
            tc.tile_pool(name="sc_ps", bufs=2, space="PSUM") as sc_psp,
            tc.tile_pool(name="r_ps", bufs=1, space="PSUM") as r_psp,
            tc.tile_pool(name="o_ps", bufs=1, space="PSUM") as o_psp,
        ):
            # dm/id gate the very first scores group -> DMA them first; the
            # ones (r1) and vcorr (output fixup) consts are needed later and
            # are emitted after block 0's input DMAs (see loop below).
            dm_sb = consts.tile([128, 128], BF16)
            nc.sync.dma_start(out=dm_sb, in_=dmask_d[:, :])
            id_sb = consts.tile([128, 128], BF16)
            nc.sync.dma_start(out=id_sb, in_=ident_d[:, :])
            ones_sb = consts.tile([128, 128], BF16)
            vcorr_sb = consts.tile([128, BLOC * H * NCH], F32)

            def late_const_dmas():
                nc.sync.dma_start(out=ones_sb, in_=ones_d[:, :])
                nc.sync.dma_start(out=vcorr_sb, in_=vcorr_d[:, :])

            NB = BLOC * H
            st = [dict() for _ in range(NB)]

            def phase_scores(blk):
                """input DMAs -> scores (+causal) -> exp1, group-pipelined.

                The r1 matmuls are NOT emitted here: the PE queue is in-order,
                so an r1 op right after its group's scores would block the
                queue on exp1(g).  They go in phase_r1, after PV(n-2) has
                filled the PE while the exp1 stream drains.
                """
                bi, h = divmod(blk, H)
                s = st[blk]
                qt_sb = qkp.tile([DK, S], BF16, tag="qt")
                kt_sb = qkp.tile([DK, S], BF16, tag="kt")
                if blk == 0:
                    # block 0 gates the whole pipeline: land the pieces the
                    # first scores matmuls need (kt cols 0:128, qt cols
                    # 0:512) before the bulk arrives
                    nc.sync.dma_start(out=kt_sb[:, 0:128], in_=kt_d[bi, h, :, 0:128])
                    nc.sync.dma_start(out=qt_sb[:, 0:512], in_=qt_d[bi, h, :, 0:512])
                    nc.sync.dma_start(out=qt_sb[:, 512:S], in_=qt_d[bi, h, :, 512:S])
                    nc.sync.dma_start(out=kt_sb[:, 128:S], in_=kt_d[bi, h, :, 128:S])
                else:
                    nc.sync.dma_start(out=qt_sb, in_=qt_d[bi, h])
                    nc.sync.dma_start(out=kt_sb, in_=kt_d[bi, h])
                vc_sb = vcp.tile([128, NCH * 128], BF16, tag="vc")
                nc.sync.dma_start(out=vc_sb, in_=vcat_d[bi, h])
                s["vc"] = vc_sb
                e1 = e1p.tile([128, PACK], BF16, tag="e1")
                s["e1"] = e1
                for grp in GROUPS:
                    gw = sum(LIVE[c] for c in grp)
                    sps = sc_psp.tile([128, S], F32, tag="sc")
                    loc = 0
                    for c in grp:
                        q0 = 128 * c
                        for n0 in range(0, LIVE[c], 512):
                            w = min(512, LIVE[c] - n0)
                            nc.tensor.matmul(
                                sps[:, loc + n0 : loc + n0 + w],
                                lhsT=kt_sb[:, q0 : q0 + 128],
                                rhs=qt_sb[:, q0 + n0 : q0 + n0 + w],
                                start=True,
                                stop=False,
                                skip_group_check=True,
                            )
                        # causal: += I^T @ dmask adds -1e30 above diag
                        nc.tensor.matmul(
                            sps[:, loc : loc + 128],
                            lhsT=id_sb,
                            rhs=dm_sb,
                            start=False,
                            stop=True,
                            skip_group_check=True,
                        )
                        loc += LIVE[c]
                    # E1T = exp(s/8) for the whole group -> packed e1
                    o0 = OFF[grp[0]]
                    nc.scalar.activation(
                        e1[:, o0 : o0 + gw], sps[:, 0:gw], AF.Exp, scale=0.125
                    )

            def phase_r1(blk):
                """r1 column sums -> rec1 (all groups' exp1 already queued)"""
                s = st[blk]
                e1 = s["e1"]
                r1ps = r_psp.tile([128, S], F32, tag="r1")
                for c in range(NCH):
                    q0 = 128 * c
                    for p0, p1 in bank_pieces(q0, S):
                        nc.tensor.matmul(
                            r1ps[:, p0:p1],
                            lhsT=ones_sb,
                            rhs=e1[:, OFF[c] + p0 - q0 : OFF[c] + p1 - q0],
                            start=(c == 0),
                            stop=(c == NCH - 1),
                            skip_group_check=True,
                        )
                rec1 = rcp.tile([128, S], F32, tag="rec1")
                nc.vector.reciprocal_approx_fast(out=rec1, in_=r1ps[:, 0:S])
                nc.vector.memset(rec1[:, 0:1], 0.0)
                s["rec1"] = rec1

            def phase_C_mul(blk, dve_chunks=None):
                """p1 = e1 * rec1 (f32, split across DVE and GpSimd)"""
                s = st[blk]
                e1, rec1 = s["e1"], s["rec1"]
                tmp = tmpp.tile([128, PACK], F32, tag="tmp")
                s["tmp"] = tmp
                dve = MUL_ON_DVE if dve_chunks is None else dve_chunks
                for c in range(NCH):
                    q0 = 128 * c
                    eng = nc.vector if c in dve else nc.gpsimd
                    eng.tensor_mul(
                        tmp[:, OFF[c] : OFF[c] + LIVE[c]],
                        e1[:, OFF[c] : OFF[c] + LIVE[c]],
                        rec1[:, q0:S],
                    )

            def phase_C_expa(blk):
                """exp of chunks 0-1 (own tile, gated only on the DVE muls)"""
                s = st[blk]
                e2a = e2p.tile([128, CSPLIT], BF16, tag="e2a")
                nc.scalar.activation(e2a, s["tmp"][:, 0:CSPLIT], AF.Exp)
                s["e2a"] = e2a

            def phase_C_expb(blk):
                """exp of chunks 2-7 (gated on the GpSimd muls)"""
                s = st[blk]
                e2b = e2p.tile([128, PACK - CSPLIT], BF16, tag="e2b")
                nc.scalar.activation(e2b, s["tmp"][:, CSPLIT:PACK], AF.Exp)
                s["e2b"] = e2b

            def phase_DE(blk):
                """dense PV sweep -> per-chunk vcorr add -> store"""
                bi, h = divmod(blk, H)
                s = st[blk]
                e2a, e2b, vc_sb = s["e2a"], s["e2b"], s["vc"]
                otps = o_psp.tile([128, S], F32, tag="ot")
                for c in range(NCH):
                    q0 = 128 * c
                    e2, eoff = (e2a, OFF[c]) if c < 2 else (e2b, OFF[c] - CSPLIT)
                    for p0, p1 in bank_pieces(q0, S):
                        nc.tensor.matmul(
                            otps[:, p0:p1],
                            lhsT=vc_sb[:, 128 * c : 128 * (c + 1)],
                            rhs=e2[:, eoff + p0 - q0 : eoff + p1 - q0],
                            start=(c == 0),
                            stop=(c == NCH - 1),
                            skip_group_check=True,
                        )
                ot_sb = outp.tile([128, S], F32, tag="otsb")
                nc.vector.memset(ot_sb[:, 0:1], 0.0)
                for c in range(NCH):
                    q0 = 128 * c
                    a0 = 1 if c == 0 else q0      # col 0 stays zero
                    nc.vector.tensor_scalar_add(
                        ot_sb[:, a0 : q0 + 128],
                        otps[:, a0 : q0 + 128],
                        vcorr_sb[:, blk * NCH + c : blk * NCH + c + 1],
                    )
                nc.sync.dma_start(
                    out=out1_d[bi, DK * h : DK * (h + 1), :], in_=ot_sb[0:DK, :]
                )
                nc.sync.dma_start(
                    out=out2_d[bi, DK * h : DK * (h + 1), :],
                    in_=ot_sb[DK : 2 * DK, :],
                )

            # 3-block-skew software pipeline, per iteration n:
            #   DVE:    muls(n-1) | tsa(n-3) | rec1(n)   (in queue order)
            #   GpSimd: muls(n-1)
            #   PE:     scores(n) | PV(n-3) | r1(n)      (PV plugs the gap
            #           while the exp1(n) stream drains on Scalar; e2(n-3)
            #           has been ready for two full iterations)
            #   Scalar: exp1(n) x6 | exp2(n-1) x2
            for n in range(NB + 3):
                if 1 <= n <= NB:
                    phase_C_mul(n - 1)
                if n < NB:
                    phase_scores(n)
                if n == 0:
                    late_const_dmas()
                if n >= 3:
                    phase_DE(n - 3)
                if n < NB:
                    phase_r1(n)
                if 1 <= n <= NB:
                    phase_C_expa(n - 1)
                    phase_C_expb(n - 1)

    nc.compile()
    return nc


_NC_CACHE = None


def _get_nc():
    global _NC_CACHE
    if _NC_CACHE is None:
        _NC_CACHE = build_nc()
    return _NC_CACHE


def make_in_maps(q, k, v1, v2, cm):
    """Full inputs -> per-core input maps (host-side sharding + layout)."""
    q = np.asarray(q, dtype=np.float32).astype(NPBF16)
    k = np.asarray(k, dtype=np.float32).astype(NPBF16)
    v1 = np.asarray(v1, dtype=np.float32)
    v2 = np.asarray(v2, dtype=np.float32)
    cm = np.asarray(cm)

    # additive causal mask for the diagonal block: 0 where k < q else -1e30
    dmask = np.where(
        np.arange(128)[:, None] < np.arange(128)[None, :], 0.0, -1e30
    ).astype(NPBF16)
    ident = np.eye(128, dtype=NPBF16)
    onesd = np.ones((128, 128), NPBF16)
    inv_s = 1.0 / S

    in_maps = []
    for core in range(NCORES):
        b0 = core * BLOC
        qt = np.ascontiguousarray(
            q[b0 : b0 + BLOC].reshape(BLOC, S, H, DK).transpose(0, 2, 3, 1)
        )  # [b, h, dk, s]
        kt = np.ascontiguousarray(
            k[b0 : b0 + BLOC].reshape(BLOC, S, H, DK).transpose(0, 2, 3, 1)
        )
        cmv = cm[b0 : b0 + BLOC].astype(np.float32)      # [b, s] 1 = masked
        cml = 1.0 - cmv
        v1s = v1[b0 : b0 + BLOC].reshape(BLOC, NCH, 128, H, DK).transpose(0, 3, 1, 2, 4)
        v2s = v2[b0 : b0 + BLOC].reshape(BLOC, NCH, 128, H, DK).transpose(0, 3, 1, 2, 4)
        vc = np.empty((BLOC, H, NCH, 128, 128), np.float32)
        vc[..., 0:DK] = v1s
        vc[..., DK : 2 * DK] = v2s
        # counter-mask and 1/S folded into the PV weights (x 2^-10 is exact);
        # transposed to [b, h, key-in-chunk, c, j] for contiguous DMA lines
        vcm = (vc * (cml.reshape(BLOC, 1, NCH, 128, 1) * inv_s)).astype(NPBF16)
        vcat = np.ascontiguousarray(vcm.transpose(0, 1, 3, 2, 4)).reshape(
            BLOC, H, 128, NCH * 128
        )
        # vcorr[c] = (sum_{k masked, chunk<=c} v_k + sum_{chunk>c} v_k)/S
        vmasked = (vc * cmv.reshape(BLOC, 1, NCH, 128, 1)).sum(axis=3)  # [b,h,c,d]
        vall = vc.sum(axis=3)
        masked_cum = np.cumsum(vmasked, axis=2)
        suffix = vall[:, :, ::-1].cumsum(axis=2)[:, :, ::-1]
        suffix = np.concatenate(
            [suffix[:, :, 1:], np.zeros_like(suffix[:, :, :1])], axis=2
        )
        vcorr = np.ascontiguousarray(
            ((masked_cum + suffix) * inv_s).transpose(3, 0, 1, 2).reshape(
                128, BLOC * H * NCH
            )
        )
        in_maps.append(
            dict(qt=qt, kt=kt, vcat=vcat, vcorr=vcorr,
                 dmask=dmask, ident=ident, onesd=onesd)
        )
    return in_maps


def _gather(res):
    out1 = np.concatenate(
        [r["out1t"].transpose(0, 2, 1) for r in res.results], axis=0
    )
    out2 = np.concatenate(
        [r["out2t"].transpose(0, 2, 1) for r in res.results], axis=0
    )
    return np.ascontiguousarray(out1), np.ascontiguousarray(out2)


def kernel(q, k, v1, v2, counter_attention_mask):
    global LAST_RESULTS
    in_maps = make_in_maps(q, k, v1, v2, counter_attention_mask)
    nc = _get_nc()
    res = run_bass_kernel_spmd(
        nc, in_maps, core_ids=list(range(NCORES)), trace=TRACE
    )
    LAST_RESULTS = res
    return _gather(res)
